# revision 11
# baseline (speedup 1.0000x reference)
"""Scatter-average of node features into dense [B, C, H, W] grids on 8 trn2 cores.

Strategy: data-parallel over batch (32 batches -> 4 per core). Per batch on
device: dense one-hot matmul segment-sum. For each 512-cell group g and each
128-node tile k, DVE/ACT builds OneHot[p, j] = (seg[p] == 512g + j) with one
fused tensor_scalar (subtract, is_equal) against an iota row; the PE
accumulates F_k^T @ OneHot into PSUM [65, 512] over all 64 node tiles.
Channel 65 of F is 1.0, so row 64 of the PSUM is the cell count. Output is
already channel-major: divide rows 0..63 by max(count, 1) and DMA out.
Race-free by construction (no scatter hardware involved).
"""

import numpy as np

from concourse import bacc, mybir, tile
from concourse.bass_utils import run_bass_kernel_spmd

B, N, C, H, W = 32, 8192, 64, 64, 64
NCORES = 8
BPC = B // NCORES          # 4 batches per core
CELLS = H * W              # 4096
ELEM = 128                 # 64 features + 64 replicated count channels
NTILE = N // 128           # 64 node tiles per batch
GRP = 512                  # cells per PSUM group (fp32 moving-operand max)
NGRP = CELLS // GRP        # 8 groups per batch

_cache = {}


def build_nc():
    nc = bacc.Bacc(target_bir_lowering=False)
    f32 = mybir.dt.float32
    feats = nc.declare_dram_parameter("features", [BPC, N, C], f32, isOutput=False)
    locs = nc.declare_dram_parameter("key_locs", [BPC, N, 2], mybir.dt.int32, isOutput=False)
    out = nc.declare_dram_parameter("out", [BPC, C, CELLS], f32, isOutput=True)

    with tile.TileContext(nc) as tc:
        with (
            tc.tile_pool(name="const", bufs=1) as cpool,
            tc.tile_pool(name="sbuf", bufs=2) as pool,
            tc.tile_pool(name="ohp", bufs=6) as ohp,
            tc.tile_pool(name="psum", bufs=2, space="PSUM") as psum,
        ):
            iota32 = cpool.tile([128, GRP], mybir.dt.int32)
            nc.gpsimd.iota(iota32[:], pattern=[[1, GRP]], channel_multiplier=0)
            iotaf = cpool.tile([128, GRP], f32)
            nc.vector.tensor_copy(out=iotaf[:], in_=iota32[:])

            for b in range(BPC):
                # features wrapped [128, 64 blocks, 65]: node i -> (i%128, i//128)
                ftile = pool.tile([128, NTILE * ELEM], f32, tag="ftile")
                f3 = ftile[:].rearrange("p (j e) -> p j e", e=ELEM)
                nc.sync.dma_start(
                    out=f3[:, :, 0:C],
                    in_=feats[b].rearrange("(j p) c -> p j c", p=128),
                )
                nc.vector.memset(f3[:, :, C:ELEM], 1.0)

                # seg = y*W + x as f32, node-tile layout [128, 64]
                ltile = pool.tile([128, NTILE * 2], mybir.dt.int32, tag="ltile")
                l3 = ltile[:].rearrange("p (j t) -> p j t", t=2)
                nc.sync.dma_start(
                    out=l3[:, :, :],
                    in_=locs[b].rearrange("(j p) t -> p j t", p=128),
                )
                seg32 = pool.tile([128, NTILE], mybir.dt.int32, tag="seg32")
                nc.vector.tensor_scalar(
                    out=seg32[:], in0=l3[:, :, 0], scalar1=W, scalar2=None,
                    op0=mybir.AluOpType.mult,
                )
                nc.vector.tensor_tensor(
                    out=seg32[:], in0=seg32[:], in1=l3[:, :, 1],
                    op=mybir.AluOpType.add,
                )
                segf = pool.tile([128, NTILE], f32, tag="segf")
                nc.vector.tensor_copy(out=segf[:], in_=seg32[:])

                for g in range(NGRP):
                    ps = psum.tile([ELEM, GRP], f32, tag="ps")
                    for k in range(NTILE):
                        oh = ohp.tile([128, GRP], f32, tag="oh")
                        # oh[p, j] = ((iota[j] - seg[p]) == -512g) = (seg[p] == 512g + j)
                        nc.any.tensor_scalar(
                            out=oh[:], in0=iotaf[:], scalar1=segf[:, k : k + 1],
                            scalar2=float(-GRP * g),
                            op0=mybir.AluOpType.subtract,
                            op1=mybir.AluOpType.is_equal,
                        )
                        nc.tensor.matmul(
                            out=ps[:], lhsT=f3[:, k, :], rhs=oh[:],
                            start=(k == 0), stop=(k == NTILE - 1),
                        )
                    cnt = pool.tile([64, GRP], f32, tag="cnt")
                    nc.vector.tensor_scalar(
                        out=cnt[:], in0=ps[64:128, :], scalar1=1.0, scalar2=None,
                        op0=mybir.AluOpType.max,
                    )
                    recip = pool.tile([64, GRP], f32, tag="recip")
                    nc.vector.reciprocal(out=recip[:], in_=cnt[:])
                    osb = pool.tile([64, GRP], f32, tag="osb")
                    nc.vector.tensor_tensor(
                        out=osb[:], in0=ps[0:64, :], in1=recip[:],
                        op=mybir.AluOpType.mult,
                    )
                    nc.sync.dma_start(
                        out=out[b][:, GRP * g : GRP * (g + 1)], in_=osb[:],
                    )
    nc.compile()
    return nc


def kernel(features: np.ndarray, key_locs: np.ndarray) -> np.ndarray:
    features = np.ascontiguousarray(features, dtype=np.float32)
    key_locs = np.ascontiguousarray(key_locs, dtype=np.int32)
    if "nc" not in _cache:
        _cache["nc"] = build_nc()
    nc = _cache["nc"]
    in_maps = [
        {
            "features": features[i * BPC : (i + 1) * BPC],
            "key_locs": key_locs[i * BPC : (i + 1) * BPC],
        }
        for i in range(NCORES)
    ]
    res = run_bass_kernel_spmd(nc, in_maps, list(range(NCORES)))
    outs = [res.results[i]["out"].reshape(BPC, C, H, W) for i in range(NCORES)]
    return np.concatenate(outs, axis=0)


if __name__ == "__main__":
    rng = np.random.default_rng(0)
    f = rng.standard_normal((B, N, C), dtype=np.float32)
    k = rng.integers(0, H, size=(B, N, 2)).astype(np.int32)
    o = kernel(f, k)
    print(o.shape, o.dtype)
